# revision 19
# baseline (speedup 1.0000x reference)
"""Multi-head attention (B=4, S=2048, D=1024, H=16, DK=64) on 8 TRN2 cores.

Sharding: core c = (b, g) with b = c//2 in 0..3 (data parallel on batch) and
g = c%2 (tensor parallel on heads: 8 heads / 512 d' columns per group).
Each core computes a partial output projection; the host sums the two
partials per batch (the "all-reduce" of the sharding hint, done host-side)
and adds bo + bv@Wo (the V bias commutes through attention exactly).

Per-core device algorithm (all matmul inputs bf16, fp32 PSUM accumulation):
  QT[d',q] = Wq_g^T Xq^T  (+bq via DVE per-partition add on the PSUM copy)
  KT[d',k] = same with Wk
  Vn[k,d'] = Xv Wv_g       natural layout, plus a ones column per head
                           -> V_aug [k, 65] per head (bv folded on host)
  per (q-chunk, head):
    ST[k,q]   = scores via lhsT=KT slice, rhs=QT slice (head pairs run in
                disjoint PE row groups)
    P = exp(ST/8)          one ACT pass per k-tile x 2 heads (PSUM->SBUF)
    AT_aug    = sum_k V_aug^T P  -> [65, q]: rows 0..63 = V^T P,
                                    row 64 = softmax denominators
    r = 1/AT_aug[64]       DVE reciprocal_approx_fast
    ATn = AT_aug[0:64] * r  (r broadcast via DRAM-bounce DMA in steady
                             state; via a tiny fp32 PE matmul in the tail)
  out[q,e] += sum_h ATn_h^T Wo_h   partial output projection (fp32 out)
"""

import os
import sys
import time
import types

sys.path.insert(0, "/opt/trn_rl_repo")

import numpy as np
import ml_dtypes

# ---------------------------------------------------------------------------
# axon NTFF profile hook (missing from this image's antenv stub); harmless
# when tracing is disabled.
# ---------------------------------------------------------------------------
def _install_axon_hooks():
    import antenv

    if "antenv.axon_hooks" in sys.modules:
        return
    hooks = types.ModuleType("antenv.axon_hooks")
    hooks._hook = None
    hooks.set_axon_ntff_profile_hook = lambda h: setattr(hooks, "_hook", h)
    hooks.get_axon_ntff_profile_hook = lambda: hooks._hook
    sys.modules["antenv.axon_hooks"] = hooks
    antenv.axon_hooks = hooks
    try:
        from trn_agent_boot.trn_boot import _ntff_profile_via_ctypes

        hooks.set_axon_ntff_profile_hook(
            _ntff_profile_via_ctypes("/opt/axon/libaxon_pjrt.so")
        )
    except Exception:
        pass


_install_axon_hooks()

import concourse.bacc as bacc
import concourse.bass as bass
import concourse.tile as tile
from concourse import mybir
from concourse import bass_utils
from concourse.bass_utils import run_bass_kernel_spmd

# The trace path uploads artifacts to a network bucket; keep it local.
bass_utils.upload_artifacts = lambda tmpdir: tmpdir

BF16 = mybir.dt.bfloat16
F32 = mybir.dt.float32

# Problem dims (hardcoded per spec)
B, S, D = 4, 2048, 1024
H, DK = 16, 64
N_CORES = 8
HC = H // N_CORES * B  # heads per core = 8  (16 heads / 2 groups)
DPC = HC * DK  # d' columns per core = 512

LAST_EXEC_TIME_NS = None


def build_program(s=S, dm=D, hc=HC, e=D):
    """Build the per-core Bass program. All dims in units of elements.

    s: sequence length (multiple of 512), dm: model dim (multiple of 128),
    hc: heads per core (even), e: output model dim (multiple of 512).
    """
    dk = DK
    dpc = hc * dk  # d' per core
    pairs = hc // 2
    dt_n = dm // 128  # d-tiles (contraction tiles for projections)
    st_n = s // 128  # s-tiles = k-tiles
    qc_n = s // 512  # q-chunks
    ec_n = e // 512  # out-proj column chunks

    nc = bacc.Bacc("TRN2", target_bir_lowering=False, debug=False,
                   num_devices=N_CORES)

    xqT = nc.dram_tensor("xqT", [dm, s], BF16, kind="ExternalInput")
    xkT = nc.dram_tensor("xkT", [dm, s], BF16, kind="ExternalInput")
    xvT = nc.dram_tensor("xvT", [dm, s], BF16, kind="ExternalInput")
    wq = nc.dram_tensor("wq", [dm, dpc], BF16, kind="ExternalInput")
    wk = nc.dram_tensor("wk", [dm, dpc], BF16, kind="ExternalInput")
    wv = nc.dram_tensor("wv", [dm, dpc], BF16, kind="ExternalInput")
    wo = nc.dram_tensor("wo", [dpc, e], BF16, kind="ExternalInput")
    bq = nc.dram_tensor("bq", [dpc], F32, kind="ExternalInput")
    bk = nc.dram_tensor("bk", [dpc], F32, kind="ExternalInput")
    out = nc.dram_tensor("out", [s, e], F32, kind="ExternalOutput")

    with tile.TileContext(nc) as tc:
        with (
            tc.tile_pool(name="singles", bufs=1) as singles,
            tc.tile_pool(name="xin", bufs=2) as xin,
            tc.tile_pool(name="expst", bufs=6) as expst_pool,
            tc.tile_pool(name="atn", bufs=hc + 4) as atn_pool,
            tc.tile_pool(name="small", bufs=2) as small,
            tc.tile_pool(name="outsb", bufs=3) as outsb_pool,
            tc.tile_pool(name="ps_sc", bufs=2, space="PSUM") as ps_sc,
            tc.tile_pool(name="ps_at", bufs=4, space="PSUM") as ps_at,
            tc.tile_pool(name="dramb", bufs=4, space="DRAM") as dramb,
        ):
            # ---- persistent SBUF tensors ----
            qt_sb = singles.tile([128, pairs, s], BF16, tag="qt")
            kt_sb = singles.tile([128, pairs, s], BF16, tag="kt")
            vn_sb = singles.tile([128, st_n, hc, dk + 1], BF16, tag="vn")
            wq_sb = singles.tile([128, dt_n, dpc], BF16, tag="wq")
            wk_sb = singles.tile([128, dt_n, dpc], BF16, tag="wk")
            wv_sb = singles.tile([128, dt_n, dpc], BF16, tag="wv")
            wo_sb = singles.tile([128, pairs, e], BF16, tag="wo")
            bq_sb = singles.tile([128, pairs], F32, tag="bq")
            bk_sb = singles.tile([128, pairs], F32, tag="bk")
            ones_sb = singles.tile([128, 512], BF16, tag="ones")
            ones32_sb = singles.tile([128, 64], F32, tag="ones32")

            # ones first: unblocks PE warm-up matmuls with no DMA dependency
            nc.vector.memset(ones_sb, 1.0)
            nc.vector.memset(ones32_sb, 1.0)
            # ones column of every V_aug head block
            nc.vector.memset(vn_sb[:, :, :, dk : dk + 1], 1.0)

            # PE warm-up: the HAM clock gate defaults to 1.2 GHz and takes
            # ~3.4us of sustained activity to release to 2.4 GHz. Junk
            # matmuls during the initial DMA wait warm it so the real
            # projection stream starts at full clock.
            warm_ps = ps_sc.tile([128, 1024], F32, tag="sc")
            for _ in range(24):
                nc.tensor.matmul(
                    warm_ps[:, 0:256], ones_sb[0:1, 0:128], ones_sb[0:1, 0:256],
                    start=True, stop=True,
                )

            # ---- input DMAs: wv + first xv blocks first (V proj starts the
            # PE pipeline), split across the two HWDGE queues (sync + scalar)
            # so issue overhead doesn't serialize the prologue. bq/bk go on
            # the gpsimd SWDGE queue (many tiny descriptors, needed late).
            nc.scalar.dma_start(
                out=wv_sb, in_=wv.ap().rearrange("(t p) n -> p t n", p=128))

            xb_v = max(1, s // 256)   # V reads 128-wide slices
            xb_p = max(1, s // 512)   # projections read 512-wide slices

            def load_x_blocked(xdram, nblk, queues=None):
                x_sb = xin.tile([128, nblk, dt_n, s // nblk], BF16, tag="x")
                src = xdram.ap().rearrange("(t p) n -> p t n", p=128)
                for j in range(nblk):
                    jsl = slice(j * s // nblk, (j + 1) * s // nblk)
                    eng = queues[j % len(queues)] if queues else nc.sync
                    eng.dma_start(out=x_sb[:, j, :, :], in_=src[:, :, jsl])
                return x_sb

            def xslice(x_sb, t, lo, width):
                nblk = x_sb.shape[1]
                bw = s // nblk
                j, off = lo // bw, lo % bw
                assert off + width <= bw
                return x_sb[:, j, t, off : off + width]

            xv_sb = load_x_blocked(xvT, xb_v, queues=[nc.sync, nc.scalar])
            # Warm-up exp ACT (emitted after the scalar queue's critical
            # prologue DMAs so it doesn't delay wv): forces the ~1.3us
            # ACT_TABLE_LOAD to overlap the projections instead of stalling
            # the first attention exp.
            warm_sb = singles.tile([128, 32], F32, tag="warm")
            nc.scalar.activation(
                warm_sb, ones_sb[:, 0:32], mybir.ActivationFunctionType.Exp)
            nc.gpsimd.dma_start(
                out=bq_sb, in_=bq.ap().rearrange("(a p) -> p a", p=128))
            nc.gpsimd.dma_start(
                out=bk_sb, in_=bk.ap().rearrange("(a p) -> p a", p=128))
            nc.sync.dma_start(
                out=wk_sb, in_=wk.ap().rearrange("(t p) n -> p t n", p=128))
            xk_sb = load_x_blocked(xkT, xb_p, queues=[nc.scalar, nc.sync])
            nc.scalar.dma_start(
                out=wq_sb, in_=wq.ap().rearrange("(t p) n -> p t n", p=128))
            nc.sync.dma_start(
                out=wo_sb, in_=wo.ap().rearrange("(a p) e -> p a e", p=128))

            # ---- helper closures ----
            def proj_qk_pair(w_sb, b_sb, x_sb, dst, p, qc):
                """One [d' 128, q 512] projection chain for a head pair."""
                ps = ps_at.tile([128, 512], F32, tag="ps")
                for t in range(dt_n):
                    nc.tensor.matmul(
                        ps,
                        w_sb[:, t, p * 128 : (p + 1) * 128],
                        xslice(x_sb, t, qc * 512, 512),
                        start=(t == 0),
                        stop=(t == dt_n - 1),
                    )
                nc.vector.tensor_scalar_add(
                    dst[:, p, qc * 512 : (qc + 1) * 512], ps, b_sb[:, p : p + 1])

            # ---- stage A upfront ----
            # V first (its DMAs were issued first); most of K/Q projection is
            # deferred into B(0) as PE filler.
            vw = min(512, dpc)
            for st in range(st_n):
                for nchunk in range(dpc // vw):
                    nsl = slice(nchunk * vw, (nchunk + 1) * vw)
                    ps = ps_at.tile([128, vw], F32, tag="ps")
                    for t in range(dt_n):
                        nc.tensor.matmul(
                            ps,
                            xslice(xv_sb, t, st * 128, 128),
                            wv_sb[:, t, nsl],
                            start=(t == 0),
                            stop=(t == dt_n - 1),
                        )
                    nc.vector.tensor_copy(
                        vn_sb[
                            :, st,
                            nchunk * (vw // dk) : (nchunk + 1) * (vw // dk),
                            0:dk,
                        ],
                        ps.rearrange("p (h d) -> p h d", d=dk),
                    )

            # xq reuses xv's slot (frees after the last V matmul)
            xq_sb = load_x_blocked(xqT, xb_p, queues=[nc.sync, nc.scalar])
            # KT pairs 0..2 upfront; pair 3 + QT0 pairs 1..3 fill B(0)
            for p in range(min(3, pairs)):
                for qcc in range(qc_n):
                    proj_qk_pair(wk_sb, bk_sb, xk_sb, kt_sb, p, qcc)
            proj_qk_pair(wq_sb, bq_sb, xq_sb, qt_sb, 0, 0)

            # ---- filler generators (drip-fed PE work) ----
            def proj_qk_gen(w_sb, b_sb, x_sb, dst, p, qcc):
                """Projection chain yielding after each matmul."""
                ps = ps_at.tile([128, 512], F32, tag="ps")
                for t in range(dt_n):
                    nc.tensor.matmul(
                        ps,
                        w_sb[:, t, p * 128 : (p + 1) * 128],
                        xslice(x_sb, t, qcc * 512, 512),
                        start=(t == 0),
                        stop=(t == dt_n - 1),
                    )
                    yield
                nc.vector.tensor_scalar_add(
                    dst[:, p, qcc * 512 : (qcc + 1) * 512], ps,
                    b_sb[:, p : p + 1])
                yield

            def outproj_gen(atn_q, qcc, qt_i, ecc):
                """Out-projection sequence (pair-packed, K=128 per matmul)."""
                esl = slice(ecc * 512, (ecc + 1) * 512)
                q0 = qcc * 4 + qt_i
                o_ps = ps_at.tile([128, 512], F32, tag="ps")
                for p in range(pairs):
                    nc.tensor.matmul(
                        o_ps,
                        atn_q[p][:, qt_i * 128 : (qt_i + 1) * 128],
                        wo_sb[:, p, esl],
                        start=(p == 0),
                        stop=(p == pairs - 1),
                    )
                    yield
                o_sb = outsb_pool.tile([128, 512], F32, tag="o")
                nc.vector.tensor_copy(o_sb, o_ps)
                nc.sync.dma_start(
                    out=out.ap()[q0 * 128 : (q0 + 1) * 128, esl], in_=o_sb)
                yield

            class FillerQueue:
                def __init__(self):
                    self.tasks = []  # (gen, deadline_pr or None)

                def add(self, gen, deadline=None):
                    self.tasks.append((gen, deadline))

                def pump(self, n):
                    while n > 0 and self.tasks:
                        try:
                            next(self.tasks[0][0])
                            n -= 1
                        except StopIteration:
                            self.tasks.pop(0)

                def fence(self, pr):
                    # complete every task whose deadline is <= pr (FIFO order
                    # matches deadline order)
                    while self.tasks and any(
                        dl is not None and dl <= pr for _, dl in self.tasks
                    ):
                        self.pump(1000)

                def drain(self):
                    while self.tasks:
                        self.pump(1000)

            # ---- stages B+C interleaved over q-chunks ----
            # B(qc) processes HEAD PAIRS: the two heads' score matmuls run
            # concurrently in different PE row groups (K=64 each); one
            # [128, 1024] exp ACT covers both heads of one k-tile. Filler
            # matmuls (C(qc-1) out-projections, QT(qc+1) projections) are
            # drip-fed between groups to keep the PE dense for HAM.
            prev_atn = None
            pending_norm = None
            for qc in range(qc_n):
                qsl = slice(qc * 512, (qc + 1) * 512)
                last = qc == qc_n - 1
                atn_q = []
                rs_sb = small.tile([hc, 512], F32, tag="rs")

                fill = FillerQueue()
                if qc == 0:
                    for pp in range(1, pairs):
                        fill.add(
                            proj_qk_gen(wq_sb, bq_sb, xq_sb, qt_sb, pp, 0),
                            deadline=pp,
                        )
                        if pairs == 4 and pp < 4:
                            fill.add(
                                proj_qk_gen(wk_sb, bk_sb, xk_sb, kt_sb, 3, pp - 1),
                                deadline=3,
                            )
                    if pairs == 4:
                        fill.add(
                            proj_qk_gen(wk_sb, bk_sb, xk_sb, kt_sb, 3, 3),
                            deadline=3,
                        )
                # QT projections first: they are never gated, while the
                # out-projections of qc-1 wait on its normalization chain --
                # a stalled filler matmul blocks the whole PE FIFO.
                if qc + 1 < qc_n:
                    for pp in range(pairs):
                        fill.add(proj_qk_gen(
                            wq_sb, bq_sb, xq_sb, qt_sb, pp, qc + 1))
                if last and pending_norm is not None:
                    # no QT fillers ahead of the out-projections in the last
                    # q-chunk: the normalization must be emitted before them
                    pending_norm()
                    pending_norm = None
                # out-projection filler assignment, rebalanced so the last
                # q-chunk (which has no QT fillers) still has enough PE work
                # to stay ahead of the exp stream: C(qc0) fully in qc1,
                # C(qc1) split 5 in qc2 + 3 in qc3, C(qc2) 6 in qc3 + 2 in
                # the tail, C(qc3) in the tail.
                seq_total = 4 * ec_n
                if last and qc_n >= 3:
                    for sq in range(seq_total - 3, seq_total):
                        fill.add(outproj_gen(
                            prev_prev_atn, qc - 2, sq // ec_n, sq % ec_n))
                if prev_atn is not None:
                    if last:
                        n_seq = seq_total - 2
                    elif qc == qc_n - 2:
                        n_seq = seq_total - 3
                    else:
                        n_seq = seq_total
                    for sq in range(n_seq):
                        fill.add(outproj_gen(
                            prev_atn, qc - 1, sq // ec_n, sq % ec_n))

                def make_emit(at_A, at_B, pr):
                    def emit_at(kk, e_sb):
                        nc.tensor.matmul(
                            at_A,
                            vn_sb[:, kk, 2 * pr, :],
                            e_sb[:, 0:512],
                            start=(kk == 0),
                            stop=(kk == st_n - 1),
                        )
                        nc.tensor.matmul(
                            at_B,
                            vn_sb[:, kk, 2 * pr + 1, :],
                            e_sb[:, 512:1024],
                            start=(kk == 0),
                            stop=(kk == st_n - 1),
                        )
                    return emit_at

                def make_finalize(at_A, at_B, pr):
                    def finalize():
                        nonlocal pending_norm
                        den_dram = None
                        if last:
                            # denominator rows first: their DMA chain is the
                            # critical path of the tail normalization, so it
                            # launches before the atn copies. Bounce DMAs run
                            # on the scalar queue (idle after the last exp),
                            # and the final pair's row copies run on the ACT
                            # engine itself (also idle by then) so they don't
                            # queue behind the DVE's atn copies.
                            den_dram = dramb.tile([2, 512], F32, tag="dend")
                            for i, at_ps in enumerate((at_A, at_B)):
                                rs_row = small.tile(
                                    [65, 512], F32, tag="rsrow")
                                if pr == pairs - 1:
                                    nc.scalar.copy(
                                        rs_row[64:65, :], at_ps[64:65, :])
                                else:
                                    nc.vector.tensor_copy(
                                        rs_row[64:65, :], at_ps[64:65, :])
                                nc.scalar.dma_start(
                                    out=den_dram[i : i + 1, :],
                                    in_=rs_row[64:65, :])
                        # pair tile: head A on partitions 0-63 (direct DVE
                        # copy), head B shifted to 64-127 via SBUF->SBUF DMA
                        atn_pair = atn_pool.tile([128, 512], BF16, tag="atn")
                        nc.vector.tensor_copy(
                            atn_pair[0:64, :], at_A[0:64, :])
                        btmp = small.tile([64, 512], BF16, tag="btmp")
                        nc.vector.tensor_copy(btmp, at_B[0:64, :])
                        nc.sync.dma_start(
                            out=atn_pair[64:128, :], in_=btmp)
                        atn_q.append(atn_pair)
                        if pr == 0 and pending_norm is not None:
                            pending_norm()
                            pending_norm = None
                        if not last:
                            for h, at_ps in (
                                (2 * pr, at_A),
                                (2 * pr + 1, at_B),
                            ):
                                rs_row = small.tile(
                                    [65, 512], F32, tag="rsrow")
                                nc.vector.tensor_copy(
                                    rs_row[64:65, :], at_ps[64:65, :])
                                nc.sync.dma_start(
                                    out=rs_sb[h : h + 1, :],
                                    in_=rs_row[64:65, :])
                        else:
                            # per-pair normalization, pipelined under later
                            # pairs and kept entirely off the PE FIFO (a PE
                            # instruction waiting on this chain would
                            # head-block every later matmul). Broadcast the
                            # DENOMINATORS via a DRAM bounce into a [128,512]
                            # pair tile, then one base-0
                            # reciprocal_approx_fast + one gpsimd multiply
                            # cover both heads.
                            bc_sb = small.tile([128, 512], F32, tag="bc")
                            for i, lo in ((0, 0), (1, 64)):
                                row = den_dram[i : i + 1, :]
                                bcast_src = bass.AP(
                                    tensor=row.tensor,
                                    offset=row.offset,
                                    ap=[[0, 64]] + list(row.ap[1:]),
                                )
                                nc.scalar.dma_start(
                                    out=bc_sb[lo : lo + 64, :],
                                    in_=bcast_src)
                            rec_bc = small.tile([128, 512], F32, tag="recbc")
                            nc.vector.reciprocal_approx_fast(
                                out=rec_bc, in_=bc_sb)
                            nc.gpsimd.tensor_mul(atn_pair, atn_pair, rec_bc)
                    return finalize

                # AT matmuls trail their exp by two k-tiles and flush into
                # the NEXT pair's first iterations, so no AT ever waits on a
                # just-issued exp (the old per-pair flush exposed one
                # exp-latency stall per pair).
                pipe = []  # (emit_fn, kk, e_sb, finalize_or_None)

                def flush_one():
                    emit_fn, kk2, e_sb2, fin = pipe.pop(0)
                    emit_fn(kk2, e_sb2)
                    if fin is not None:
                        fin()

                for pr in range(pairs):
                    fill.fence(pr)
                    at_A = ps_at.tile([65, 512], F32, tag="ps")
                    at_B = ps_at.tile([65, 512], F32, tag="ps")
                    emit_fn = make_emit(at_A, at_B, pr)
                    fin_fn = make_finalize(at_A, at_B, pr)

                    for kk in range(st_n):
                        sc_ps = ps_sc.tile([128, 1024], F32, tag="sc")
                        ksl = slice(kk * 128, (kk + 1) * 128)
                        nc.tensor.matmul(
                            sc_ps[:, 0:512],
                            kt_sb[0:64, pr, ksl],
                            qt_sb[0:64, pr, qsl],
                            start=True,
                            stop=True,
                        )
                        nc.tensor.matmul(
                            sc_ps[:, 512:1024],
                            kt_sb[64:128, pr, ksl],
                            qt_sb[64:128, pr, qsl],
                            start=True,
                            stop=True,
                        )
                        exp_sb = expst_pool.tile([128, 1024], BF16, tag="e")
                        nc.scalar.activation(
                            exp_sb, sc_ps,
                            mybir.ActivationFunctionType.Exp,
                            scale=1.0 / np.sqrt(dk),
                        )
                        if len(pipe) >= 2:
                            flush_one()
                        pipe.append((
                            emit_fn, kk, exp_sb,
                            fin_fn if kk == st_n - 1 else None,
                        ))
                        if last and pr == 0 and kk < 4:
                            pass  # let the qc-2 norm chain land before its
                                  # out-projections can head-block the FIFO
                        elif kk % 2 == 1:
                            # bursts of 2: consecutive filler matmuls from
                            # the same chain pipeline their LDWEIGHTS
                            fill.pump(4 if last or pr == pairs - 1 else 2)

                while pipe:
                    flush_one()
                fill.drain()

                # batched softmax normalization for the whole q-chunk; its
                # emission is deferred into the next q-chunk (after the first
                # pair unit) so the DVE/DMA latency chain never head-blocks
                # the FIFO right at the q-chunk boundary.
                if not last:
                    def make_norm(rs_sb=rs_sb, atn_q=atn_q):
                        def norm():
                            rec_sb = small.tile([hc, 512], F32, tag="rec")
                            nc.vector.reciprocal_approx_fast(
                                out=rec_sb, in_=rs_sb)
                            rec_dram = dramb.tile([hc, 512], F32, tag="recd")
                            nc.sync.dma_start(out=rec_dram, in_=rec_sb)
                            for h in range(hc):
                                p, lo = h // 2, 64 * (h % 2)
                                prng = slice(lo, lo + 64)
                                row = rec_dram[h : h + 1, :]
                                bc_sb = small.tile([128, 512], F32, tag="bc")
                                bcast_src = bass.AP(
                                    tensor=row.tensor,
                                    offset=row.offset,
                                    ap=[[0, 64]] + list(row.ap[1:]),
                                )
                                nc.sync.dma_start(
                                    out=bc_sb[prng, :], in_=bcast_src)
                                nc.gpsimd.tensor_mul(
                                    atn_q[p][prng, :], atn_q[p][prng, :],
                                    bc_sb[prng, :])
                        return norm
                    pending_norm = make_norm()
                prev_prev_atn = prev_atn
                prev_atn = atn_q

            # tail: the deferred C(qc_n-2) sequences are fully ready; run
            # them first. The C(qc_n-1) sequences split into a READY phase
            # (pairs 0..2, whose norms landed during the last q-chunk;
            # partial sums parked in SBUF) and a GATED phase (one pair-3
            # matmul + DVE add each). All 24 ready matmuls emit before any
            # instruction gated on the final pair's normalization chain, so
            # the PE stays busy under it and only ~2us of gated work
            # remains at the very end.
            if qc_n >= 2:
                for sq in range(4 * ec_n - 2, 4 * ec_n):
                    for _ in outproj_gen(
                        prev_prev_atn, qc_n - 2, sq // ec_n, sq % ec_n
                    ):
                        pass
            tail_idx = [(qt_i, ecc) for qt_i in range(4) for ecc in range(ec_n)]
            partials = []
            for qt_i, ecc in tail_idx:
                esl = slice(ecc * 512, (ecc + 1) * 512)
                o_ps = ps_at.tile([128, 512], F32, tag="ps")
                for p in range(pairs - 1):
                    nc.tensor.matmul(
                        o_ps,
                        prev_atn[p][:, qt_i * 128 : (qt_i + 1) * 128],
                        wo_sb[:, p, esl],
                        start=(p == 0),
                        stop=(p == pairs - 2),
                    )
                part = outsb_pool.tile([128, 512], BF16, tag="part", bufs=8)
                nc.vector.tensor_copy(part, o_ps)
                partials.append(part)
            for (qt_i, ecc), part in zip(tail_idx, partials):
                esl = slice(ecc * 512, (ecc + 1) * 512)
                q0 = (qc_n - 1) * 4 + qt_i
                o_ps = ps_at.tile([128, 512], F32, tag="ps")
                nc.tensor.matmul(
                    o_ps,
                    prev_atn[pairs - 1][:, qt_i * 128 : (qt_i + 1) * 128],
                    wo_sb[:, pairs - 1, esl],
                    start=True,
                    stop=True,
                )
                o_sb = outsb_pool.tile([128, 512], F32, tag="o")
                nc.vector.tensor_add(o_sb, o_ps, part)
                nc.sync.dma_start(
                    out=out.ap()[q0 * 128 : (q0 + 1) * 128, esl], in_=o_sb)


    nc.compile()
    return nc


_PROGRAM_CACHE = {}


def _get_program(key):
    if key not in _PROGRAM_CACHE:
        _PROGRAM_CACHE[key] = build_program(*key)
    return _PROGRAM_CACHE[key]


def kernel(queries, keys, values, Wq, bq, Wk, bk, Wv, bv, Wo, bo):
    global LAST_EXEC_TIME_NS
    bf16 = ml_dtypes.bfloat16

    nc = _get_program((S, D, HC, D))

    xT = {}
    for name, arr in (("q", queries), ("k", keys), ("v", values)):
        xT[name] = [
            np.ascontiguousarray(np.asarray(arr[b]).T).astype(bf16)
            for b in range(B)
        ]
    Wq, Wk, Wv, Wo = (np.asarray(w) for w in (Wq, Wk, Wv, Wo))
    bqv, bkv, bvv = (np.asarray(v, dtype=np.float32) for v in (bq, bk, bv))

    in_maps = []
    for c in range(N_CORES):
        b, g = c // 2, c % 2
        csl = slice(g * DPC, (g + 1) * DPC)
        in_maps.append(
            {
                "xqT": xT["q"][b],
                "xkT": xT["k"][b],
                "xvT": xT["v"][b],
                "wq": np.ascontiguousarray(Wq[:, csl]).astype(bf16),
                "wk": np.ascontiguousarray(Wk[:, csl]).astype(bf16),
                "wv": np.ascontiguousarray(Wv[:, csl]).astype(bf16),
                "wo": np.ascontiguousarray(Wo[csl, :]).astype(bf16),
                "bq": np.ascontiguousarray(bqv[csl]),
                "bk": np.ascontiguousarray(bkv[csl]),
            }
        )

    trace = os.environ.get("KERNEL_TRACE", "0") == "1"
    res = run_bass_kernel_spmd(nc, in_maps, list(range(N_CORES)), trace=trace)
    LAST_EXEC_TIME_NS = res.exec_time_ns

    bo = np.asarray(bo, dtype=np.float32)
    # bv commutes through attention exactly: softmax rows sum to 1, so the
    # V bias contributes the constant vector bv @ Wo to every output row.
    const = bo + bvv.astype(np.float32) @ Wo.astype(np.float32)
    out = np.empty((B, S, D), dtype=np.float32)
    for b in range(B):
        out[b] = res.results[2 * b]["out"] + res.results[2 * b + 1]["out"] + const
    return out


if __name__ == "__main__":
    rng = np.random.default_rng(0)
    t0 = time.time()
    nc = _get_program((S, D, HC, D))
    print(f"build+compile: {time.time() - t0:.1f}s")


# revision 23
# speedup vs baseline: 1.0218x; 1.0218x over previous
"""Multi-head attention (B=4, S=2048, D=1024, H=16, DK=64) on 8 TRN2 cores.

Sharding: core c = (b, g) with b = c//2 in 0..3 (data parallel on batch) and
g = c%2 (tensor parallel on heads: 8 heads / 512 d' columns per group).
Each core computes a partial output projection; the host sums the two
partials per batch (the "all-reduce" of the sharding hint, done host-side)
and adds bo + bv@Wo (the V bias commutes through attention exactly).

Per-core device algorithm (all matmul inputs bf16, fp32 PSUM accumulation):
  QT[d',q] = Wq_g^T Xq^T  (+bq via DVE per-partition add on the PSUM copy)
  KT[d',k] = same with Wk
  Vn[k,d'] = Xv Wv_g       natural layout, plus a ones column per head
                           -> V_aug [k, 65] per head (bv folded on host)
  per (q-chunk, head):
    ST[k,q]   = scores via lhsT=KT slice, rhs=QT slice (head pairs run in
                disjoint PE row groups)
    P = exp(ST/8)          one ACT pass per k-tile x 2 heads (PSUM->SBUF)
    AT_aug    = sum_k V_aug^T P  -> [65, q]: rows 0..63 = V^T P,
                                    row 64 = softmax denominators
    r = 1/AT_aug[64]       DVE reciprocal_approx_fast
    ATn = AT_aug[0:64] * r  (r broadcast via DRAM-bounce DMA in steady
                             state; via a tiny fp32 PE matmul in the tail)
  out[q,e] += sum_h ATn_h^T Wo_h   partial output projection (fp32 out)
"""

import os
import sys
import time
import types

sys.path.insert(0, "/opt/trn_rl_repo")

import numpy as np
import ml_dtypes

# ---------------------------------------------------------------------------
# axon NTFF profile hook (missing from this image's antenv stub); harmless
# when tracing is disabled.
# ---------------------------------------------------------------------------
def _install_axon_hooks():
    import antenv

    if "antenv.axon_hooks" in sys.modules:
        return
    hooks = types.ModuleType("antenv.axon_hooks")
    hooks._hook = None
    hooks.set_axon_ntff_profile_hook = lambda h: setattr(hooks, "_hook", h)
    hooks.get_axon_ntff_profile_hook = lambda: hooks._hook
    sys.modules["antenv.axon_hooks"] = hooks
    antenv.axon_hooks = hooks
    try:
        from trn_agent_boot.trn_boot import _ntff_profile_via_ctypes

        hooks.set_axon_ntff_profile_hook(
            _ntff_profile_via_ctypes("/opt/axon/libaxon_pjrt.so")
        )
    except Exception:
        pass


_install_axon_hooks()

import concourse.bacc as bacc
import concourse.bass as bass
import concourse.tile as tile
from concourse import mybir
from concourse import bass_utils
from concourse.bass_utils import run_bass_kernel_spmd

# The trace path uploads artifacts to a network bucket; keep it local.
bass_utils.upload_artifacts = lambda tmpdir: tmpdir

BF16 = mybir.dt.bfloat16
F32 = mybir.dt.float32

# Problem dims (hardcoded per spec)
B, S, D = 4, 2048, 1024
H, DK = 16, 64
N_CORES = 8
HC = H // N_CORES * B  # heads per core = 8  (16 heads / 2 groups)
DPC = HC * DK  # d' columns per core = 512

LAST_EXEC_TIME_NS = None


def build_program(s=S, dm=D, hc=HC, e=D):
    """Build the per-core Bass program. All dims in units of elements.

    s: sequence length (multiple of 512), dm: model dim (multiple of 128),
    hc: heads per core (even), e: output model dim (multiple of 512).
    """
    dk = DK
    dpc = hc * dk  # d' per core
    pairs = hc // 2
    dt_n = dm // 128  # d-tiles (contraction tiles for projections)
    st_n = s // 128  # s-tiles = k-tiles
    qc_n = s // 512  # q-chunks
    ec_n = e // 512  # out-proj column chunks

    nc = bacc.Bacc("TRN2", target_bir_lowering=False, debug=False,
                   num_devices=N_CORES)

    xqT = nc.dram_tensor("xqT", [dm, s], BF16, kind="ExternalInput")
    xkT = nc.dram_tensor("xkT", [dm, s], BF16, kind="ExternalInput")
    xvT = nc.dram_tensor("xvT", [dm, s], BF16, kind="ExternalInput")
    wq = nc.dram_tensor("wq", [dm, dpc], BF16, kind="ExternalInput")
    wk = nc.dram_tensor("wk", [dm, dpc], BF16, kind="ExternalInput")
    wv = nc.dram_tensor("wv", [dm, dpc], BF16, kind="ExternalInput")
    wo = nc.dram_tensor("wo", [dpc, e], BF16, kind="ExternalInput")
    bq = nc.dram_tensor("bq", [dpc], F32, kind="ExternalInput")
    bk = nc.dram_tensor("bk", [dpc], F32, kind="ExternalInput")
    out = nc.dram_tensor("out", [s, e], F32, kind="ExternalOutput")

    with tile.TileContext(nc) as tc:
        with (
            tc.tile_pool(name="singles", bufs=1) as singles,
            tc.tile_pool(name="xin", bufs=2) as xin,
            tc.tile_pool(name="expst", bufs=6) as expst_pool,
            tc.tile_pool(name="atn", bufs=hc + 4) as atn_pool,
            tc.tile_pool(name="small", bufs=2) as small,
            tc.tile_pool(name="outsb", bufs=3) as outsb_pool,
            tc.tile_pool(name="ps_sc", bufs=2, space="PSUM") as ps_sc,
            tc.tile_pool(name="ps_at", bufs=4, space="PSUM") as ps_at,
            tc.tile_pool(name="dramb", bufs=4, space="DRAM") as dramb,
        ):
            # ---- persistent SBUF tensors ----
            qt_sb = singles.tile([128, pairs, s], BF16, tag="qt")
            kt_sb = singles.tile([128, pairs, s], BF16, tag="kt")
            vn_sb = singles.tile([128, st_n, hc, dk + 1], BF16, tag="vn")
            wq_sb = singles.tile([128, dt_n, dpc], BF16, tag="wq")
            wk_sb = singles.tile([128, dt_n, dpc], BF16, tag="wk")
            wv_sb = singles.tile([128, dt_n, dpc], BF16, tag="wv")
            wo_sb = singles.tile([128, pairs, e], BF16, tag="wo")
            bq_sb = singles.tile([128, pairs], F32, tag="bq")
            bk_sb = singles.tile([128, pairs], F32, tag="bk")
            ones_sb = singles.tile([128, 512], BF16, tag="ones")
            ones32_sb = singles.tile([128, 64], F32, tag="ones32")

            # ones first: unblocks PE warm-up matmuls with no DMA dependency
            nc.vector.memset(ones_sb, 1.0)
            nc.vector.memset(ones32_sb, 1.0)
            # ones column of every V_aug head block
            nc.vector.memset(vn_sb[:, :, :, dk : dk + 1], 1.0)

            # PE warm-up: the HAM clock gate defaults to 1.2 GHz and takes
            # ~3.4us of sustained activity to release to 2.4 GHz. Junk
            # matmuls during the initial DMA wait warm it so the real
            # projection stream starts at full clock.
            warm_ps = ps_sc.tile([128, 1024], F32, tag="sc")
            for _ in range(24):
                nc.tensor.matmul(
                    warm_ps[:, 0:256], ones_sb[0:1, 0:128], ones_sb[0:1, 0:256],
                    start=True, stop=True,
                )

            # ---- input DMAs: wv first as 4 pieces on the sync queue (the
            # scalar queue is blocked ~1.3us by the hoisted ACT_TABLE_LOAD),
            # xv blocks on the scalar queue. The first V projection needs all
            # of wv + xv block 0; everything else has slack. bq/bk go on the
            # gpsimd SWDGE queue (many tiny descriptors, needed late).
            wv_src = wv.ap().rearrange("(t p) n -> p t n", p=128)
            for i in range(4):
                tsl = slice(2 * i, 2 * i + 2)
                nc.sync.dma_start(out=wv_sb[:, tsl, :], in_=wv_src[:, tsl, :])

            xb_v = max(1, s // 256)   # V reads 128-wide slices
            xb_p = max(1, s // 512)   # projections read 512-wide slices

            def load_x_blocked(xdram, nblk, queues=None):
                x_sb = xin.tile([128, nblk, dt_n, s // nblk], BF16, tag="x")
                src = xdram.ap().rearrange("(t p) n -> p t n", p=128)
                for j in range(nblk):
                    jsl = slice(j * s // nblk, (j + 1) * s // nblk)
                    eng = queues[j % len(queues)] if queues else nc.sync
                    eng.dma_start(out=x_sb[:, j, :, :], in_=src[:, :, jsl])
                return x_sb

            def xslice(x_sb, t, lo, width):
                nblk = x_sb.shape[1]
                bw = s // nblk
                j, off = lo // bw, lo % bw
                assert off + width <= bw
                return x_sb[:, j, t, off : off + width]

            xv_sb = load_x_blocked(xvT, xb_v, queues=[nc.scalar])
            # Warm-up exp ACT (emitted after the scalar queue's critical
            # prologue DMAs so it doesn't delay wv): forces the ~1.3us
            # ACT_TABLE_LOAD to overlap the projections instead of stalling
            # the first attention exp.
            warm_sb = singles.tile([128, 32], F32, tag="warm")
            nc.scalar.activation(
                warm_sb, ones_sb[:, 0:32], mybir.ActivationFunctionType.Exp)
            nc.gpsimd.dma_start(
                out=bq_sb, in_=bq.ap().rearrange("(a p) -> p a", p=128))
            nc.gpsimd.dma_start(
                out=bk_sb, in_=bk.ap().rearrange("(a p) -> p a", p=128))
            nc.sync.dma_start(
                out=wk_sb, in_=wk.ap().rearrange("(t p) n -> p t n", p=128))
            xk_sb = load_x_blocked(xkT, xb_p, queues=[nc.scalar, nc.sync])
            nc.scalar.dma_start(
                out=wq_sb, in_=wq.ap().rearrange("(t p) n -> p t n", p=128))
            nc.sync.dma_start(
                out=wo_sb, in_=wo.ap().rearrange("(a p) e -> p a e", p=128))

            # ---- helper closures ----
            def proj_qk_pair(w_sb, b_sb, x_sb, dst, p, qc):
                """One [d' 128, q 512] projection chain for a head pair."""
                ps = ps_at.tile([128, 512], F32, tag="ps")
                for t in range(dt_n):
                    nc.tensor.matmul(
                        ps,
                        w_sb[:, t, p * 128 : (p + 1) * 128],
                        xslice(x_sb, t, qc * 512, 512),
                        start=(t == 0),
                        stop=(t == dt_n - 1),
                    )
                nc.vector.tensor_scalar_add(
                    dst[:, p, qc * 512 : (qc + 1) * 512], ps, b_sb[:, p : p + 1])

            # ---- stage A upfront ----
            # V first (its DMAs were issued first); most of K/Q projection is
            # deferred into B(0) as PE filler.
            vw = min(512, dpc)
            for st in range(st_n):
                for nchunk in range(dpc // vw):
                    nsl = slice(nchunk * vw, (nchunk + 1) * vw)
                    ps = ps_at.tile([128, vw], F32, tag="ps")
                    for t in range(dt_n):
                        nc.tensor.matmul(
                            ps,
                            xslice(xv_sb, t, st * 128, 128),
                            wv_sb[:, t, nsl],
                            start=(t == 0),
                            stop=(t == dt_n - 1),
                        )
                    nc.vector.tensor_copy(
                        vn_sb[
                            :, st,
                            nchunk * (vw // dk) : (nchunk + 1) * (vw // dk),
                            0:dk,
                        ],
                        ps.rearrange("p (h d) -> p h d", d=dk),
                    )

            # xq reuses xv's slot (frees after the last V matmul)
            xq_sb = load_x_blocked(xqT, xb_p, queues=[nc.sync, nc.scalar])
            # KT pairs 0..2 upfront; pair 3 + QT0 pairs 1..3 fill B(0)
            for p in range(min(3, pairs)):
                for qcc in range(qc_n):
                    proj_qk_pair(wk_sb, bk_sb, xk_sb, kt_sb, p, qcc)
            proj_qk_pair(wq_sb, bq_sb, xq_sb, qt_sb, 0, 0)

            # ---- filler generators (drip-fed PE work) ----
            def proj_qk_gen(w_sb, b_sb, x_sb, dst, p, qcc):
                """Projection chain yielding after each matmul."""
                ps = ps_at.tile([128, 512], F32, tag="ps")
                for t in range(dt_n):
                    nc.tensor.matmul(
                        ps,
                        w_sb[:, t, p * 128 : (p + 1) * 128],
                        xslice(x_sb, t, qcc * 512, 512),
                        start=(t == 0),
                        stop=(t == dt_n - 1),
                    )
                    yield
                nc.vector.tensor_scalar_add(
                    dst[:, p, qcc * 512 : (qcc + 1) * 512], ps,
                    b_sb[:, p : p + 1])
                yield

            def outproj_gen(atn_q, qcc, qt_i, ecc):
                """Out-projection sequence (pair-packed, K=128 per matmul)."""
                esl = slice(ecc * 512, (ecc + 1) * 512)
                q0 = qcc * 4 + qt_i
                o_ps = ps_at.tile([128, 512], F32, tag="ps")
                for p in range(pairs):
                    nc.tensor.matmul(
                        o_ps,
                        atn_q[p][:, qt_i * 128 : (qt_i + 1) * 128],
                        wo_sb[:, p, esl],
                        start=(p == 0),
                        stop=(p == pairs - 1),
                    )
                    yield
                o_sb = outsb_pool.tile([128, 512], F32, tag="o")
                nc.vector.tensor_copy(o_sb, o_ps)
                nc.sync.dma_start(
                    out=out.ap()[q0 * 128 : (q0 + 1) * 128, esl], in_=o_sb)
                yield

            class FillerQueue:
                def __init__(self):
                    self.tasks = []  # (gen, deadline_pr or None)

                def add(self, gen, deadline=None):
                    self.tasks.append((gen, deadline))

                def pump(self, n):
                    while n > 0 and self.tasks:
                        try:
                            next(self.tasks[0][0])
                            n -= 1
                        except StopIteration:
                            self.tasks.pop(0)

                def fence(self, pr):
                    # complete every task whose deadline is <= pr (FIFO order
                    # matches deadline order)
                    while self.tasks and any(
                        dl is not None and dl <= pr for _, dl in self.tasks
                    ):
                        self.pump(1000)

                def drain(self):
                    while self.tasks:
                        self.pump(1000)

            # ---- stages B+C interleaved over q-chunks ----
            # B(qc) processes HEAD PAIRS: the two heads' score matmuls run
            # concurrently in different PE row groups (K=64 each); one
            # [128, 1024] exp ACT covers both heads of one k-tile. Filler
            # matmuls (C(qc-1) out-projections, QT(qc+1) projections) are
            # drip-fed between groups to keep the PE dense for HAM.
            prev_atn = None
            pending_norm = None
            for qc in range(qc_n):
                qsl = slice(qc * 512, (qc + 1) * 512)
                last = qc == qc_n - 1
                atn_q = []
                rs_sb = small.tile([hc, 512], F32, tag="rs")

                fill = FillerQueue()
                if qc == 0:
                    for pp in range(1, pairs):
                        fill.add(
                            proj_qk_gen(wq_sb, bq_sb, xq_sb, qt_sb, pp, 0),
                            deadline=pp,
                        )
                        if pairs == 4 and pp < 4:
                            fill.add(
                                proj_qk_gen(wk_sb, bk_sb, xk_sb, kt_sb, 3, pp - 1),
                                deadline=3,
                            )
                    if pairs == 4:
                        fill.add(
                            proj_qk_gen(wk_sb, bk_sb, xk_sb, kt_sb, 3, 3),
                            deadline=3,
                        )
                # QT projections first: they are never gated, while the
                # out-projections of qc-1 wait on its normalization chain --
                # a stalled filler matmul blocks the whole PE FIFO.
                if qc + 1 < qc_n:
                    for pp in range(pairs):
                        fill.add(proj_qk_gen(
                            wq_sb, bq_sb, xq_sb, qt_sb, pp, qc + 1))
                if last and pending_norm is not None:
                    # no QT fillers ahead of the out-projections in the last
                    # q-chunk: the normalization must be emitted before them
                    pending_norm()
                    pending_norm = None
                # out-projection filler assignment, rebalanced so the last
                # q-chunk (which has no QT fillers) still has enough PE work
                # to stay ahead of the exp stream: C(qc0) fully in qc1,
                # C(qc1) split 5 in qc2 + 3 in qc3, C(qc2) 6 in qc3 + 2 in
                # the tail, C(qc3) in the tail.
                seq_total = 4 * ec_n
                if last and qc_n >= 3:
                    for sq in range(seq_total - 3, seq_total):
                        fill.add(outproj_gen(
                            prev_prev_atn, qc - 2, sq // ec_n, sq % ec_n))
                if prev_atn is not None:
                    if last:
                        n_seq = seq_total - 2
                    elif qc == qc_n - 2:
                        n_seq = seq_total - 3
                    else:
                        n_seq = seq_total
                    for sq in range(n_seq):
                        fill.add(outproj_gen(
                            prev_atn, qc - 1, sq // ec_n, sq % ec_n))

                def make_emit(at_A, at_B, pr):
                    def emit_at(kk, e_sb):
                        nc.tensor.matmul(
                            at_A,
                            vn_sb[:, kk, 2 * pr, :],
                            e_sb[:, 0:512],
                            start=(kk == 0),
                            stop=(kk == st_n - 1),
                        )
                        nc.tensor.matmul(
                            at_B,
                            vn_sb[:, kk, 2 * pr + 1, :],
                            e_sb[:, 512:1024],
                            start=(kk == 0),
                            stop=(kk == st_n - 1),
                        )
                    return emit_at

                def make_finalize(at_A, at_B, pr):
                    def finalize():
                        nonlocal pending_norm
                        den_dram = None
                        if last:
                            # denominator rows first: their DMA chain is the
                            # critical path of the tail normalization, so it
                            # launches before the atn copies. Bounce DMAs run
                            # on the scalar queue (idle after the last exp),
                            # and the final pair's row copies run on the ACT
                            # engine itself (also idle by then) so they don't
                            # queue behind the DVE's atn copies.
                            den_dram = dramb.tile([2, 512], F32, tag="dend")
                            den_q = (nc.scalar, nc.sync)
                            for i, at_ps in enumerate((at_A, at_B)):
                                rs_row = small.tile(
                                    [65, 512], F32, tag="rsrow")
                                if pr == pairs - 1:
                                    nc.scalar.copy(
                                        rs_row[64:65, :], at_ps[64:65, :])
                                else:
                                    nc.vector.tensor_copy(
                                        rs_row[64:65, :], at_ps[64:65, :])
                                den_q[i].dma_start(
                                    out=den_dram[i : i + 1, :],
                                    in_=rs_row[64:65, :])
                        # pair tile: head A on partitions 0-63 (direct DVE
                        # copy), head B shifted to 64-127 via SBUF->SBUF DMA
                        atn_pair = atn_pool.tile([128, 512], BF16, tag="atn")
                        nc.vector.tensor_copy(
                            atn_pair[0:64, :], at_A[0:64, :])
                        btmp = small.tile([64, 512], BF16, tag="btmp")
                        nc.vector.tensor_copy(btmp, at_B[0:64, :])
                        nc.sync.dma_start(
                            out=atn_pair[64:128, :], in_=btmp)
                        atn_q.append(atn_pair)
                        if pr == 0 and pending_norm is not None:
                            pending_norm()
                            pending_norm = None
                        if not last:
                            for h, at_ps in (
                                (2 * pr, at_A),
                                (2 * pr + 1, at_B),
                            ):
                                rs_row = small.tile(
                                    [65, 512], F32, tag="rsrow")
                                nc.vector.tensor_copy(
                                    rs_row[64:65, :], at_ps[64:65, :])
                                nc.sync.dma_start(
                                    out=rs_sb[h : h + 1, :],
                                    in_=rs_row[64:65, :])
                        else:
                            # per-pair normalization, pipelined under later
                            # pairs and kept entirely off the PE FIFO (a PE
                            # instruction waiting on this chain would
                            # head-block every later matmul). Broadcast the
                            # DENOMINATORS via a DRAM bounce into a [128,512]
                            # pair tile, then one base-0
                            # reciprocal_approx_fast + one gpsimd multiply
                            # cover both heads.
                            # bcast issues on sync + gpsimd: they must wait
                            # for the den DMAs' completion sems, and a
                            # waiting dma_start would head-block the scalar
                            # queue in front of the tail's ACT part-copies.
                            bc_sb = small.tile([128, 512], F32, tag="bc")
                            bc_q = (nc.sync, nc.gpsimd)
                            for i, lo in ((0, 0), (1, 64)):
                                row = den_dram[i : i + 1, :]
                                bcast_src = bass.AP(
                                    tensor=row.tensor,
                                    offset=row.offset,
                                    ap=[[0, 64]] + list(row.ap[1:]),
                                )
                                bc_q[i].dma_start(
                                    out=bc_sb[lo : lo + 64, :],
                                    in_=bcast_src)
                            rec_bc = small.tile([128, 512], F32, tag="recbc")
                            nc.vector.reciprocal_approx_fast(
                                out=rec_bc, in_=bc_sb)
                            nc.gpsimd.tensor_mul(atn_pair, atn_pair, rec_bc)
                    return finalize

                # AT matmuls trail their exp by two k-tiles and flush into
                # the NEXT pair's first iterations, so no AT ever waits on a
                # just-issued exp (the old per-pair flush exposed one
                # exp-latency stall per pair).
                pipe = []  # (emit_fn, kk, e_sb, finalize_or_None)

                def flush_one():
                    emit_fn, kk2, e_sb2, fin = pipe.pop(0)
                    emit_fn(kk2, e_sb2)
                    if fin is not None:
                        fin()

                for pr in range(pairs):
                    fill.fence(pr)
                    at_A = ps_at.tile([65, 512], F32, tag="ps")
                    at_B = ps_at.tile([65, 512], F32, tag="ps")
                    emit_fn = make_emit(at_A, at_B, pr)
                    fin_fn = make_finalize(at_A, at_B, pr)

                    for kk in range(st_n):
                        sc_ps = ps_sc.tile([128, 1024], F32, tag="sc")
                        ksl = slice(kk * 128, (kk + 1) * 128)
                        nc.tensor.matmul(
                            sc_ps[:, 0:512],
                            kt_sb[0:64, pr, ksl],
                            qt_sb[0:64, pr, qsl],
                            start=True,
                            stop=True,
                        )
                        nc.tensor.matmul(
                            sc_ps[:, 512:1024],
                            kt_sb[64:128, pr, ksl],
                            qt_sb[64:128, pr, qsl],
                            start=True,
                            stop=True,
                        )
                        exp_sb = expst_pool.tile([128, 1024], BF16, tag="e")
                        nc.scalar.activation(
                            exp_sb, sc_ps,
                            mybir.ActivationFunctionType.Exp,
                            scale=1.0 / np.sqrt(dk),
                        )
                        if len(pipe) >= 2:
                            flush_one()
                        pipe.append((
                            emit_fn, kk, exp_sb,
                            fin_fn if kk == st_n - 1 else None,
                        ))
                        if last and pr == 0 and kk < 4:
                            pass  # let the qc-2 norm chain land before its
                                  # out-projections can head-block the FIFO
                        elif kk % 2 == 1:
                            # bursts of 2: consecutive filler matmuls from
                            # the same chain pipeline their LDWEIGHTS
                            fill.pump(4 if last or pr == pairs - 1 else 2)

                while pipe:
                    flush_one()
                fill.drain()

                # batched softmax normalization for the whole q-chunk; its
                # emission is deferred into the next q-chunk (after the first
                # pair unit) so the DVE/DMA latency chain never head-blocks
                # the FIFO right at the q-chunk boundary.
                if not last:
                    def make_norm(rs_sb=rs_sb, atn_q=atn_q):
                        def norm():
                            rec_sb = small.tile([hc, 512], F32, tag="rec")
                            nc.vector.reciprocal_approx_fast(
                                out=rec_sb, in_=rs_sb)
                            rec_dram = dramb.tile([hc, 512], F32, tag="recd")
                            nc.sync.dma_start(out=rec_dram, in_=rec_sb)
                            for h in range(hc):
                                p, lo = h // 2, 64 * (h % 2)
                                prng = slice(lo, lo + 64)
                                row = rec_dram[h : h + 1, :]
                                bc_sb = small.tile([128, 512], F32, tag="bc")
                                bcast_src = bass.AP(
                                    tensor=row.tensor,
                                    offset=row.offset,
                                    ap=[[0, 64]] + list(row.ap[1:]),
                                )
                                nc.sync.dma_start(
                                    out=bc_sb[prng, :], in_=bcast_src)
                                nc.gpsimd.tensor_mul(
                                    atn_q[p][prng, :], atn_q[p][prng, :],
                                    bc_sb[prng, :])
                        return norm
                    pending_norm = make_norm()
                prev_prev_atn = prev_atn
                prev_atn = atn_q

            # tail: the deferred C(qc_n-2) sequences are fully ready; run
            # them first. The C(qc_n-1) sequences split into a READY phase
            # (pairs 0..2, whose norms landed during the last q-chunk;
            # partial sums parked in SBUF) and a GATED phase (one pair-3
            # matmul + DVE add each). All 24 ready matmuls emit before any
            # instruction gated on the final pair's normalization chain, so
            # the PE stays busy under it and only ~2us of gated work
            # remains at the very end.
            if qc_n >= 2:
                for sq in range(4 * ec_n - 2, 4 * ec_n):
                    for _ in outproj_gen(
                        prev_prev_atn, qc_n - 2, sq // ec_n, sq % ec_n
                    ):
                        pass
            tail_idx = [(qt_i, ecc) for qt_i in range(4) for ecc in range(ec_n)]
            partials = []
            for qt_i, ecc in tail_idx:
                esl = slice(ecc * 512, (ecc + 1) * 512)
                o_ps = ps_at.tile([128, 512], F32, tag="ps")
                for p in range(pairs - 1):
                    nc.tensor.matmul(
                        o_ps,
                        prev_atn[p][:, qt_i * 128 : (qt_i + 1) * 128],
                        wo_sb[:, p, esl],
                        start=(p == 0),
                        stop=(p == pairs - 2),
                    )
                part = outsb_pool.tile([128, 512], BF16, tag="part", bufs=8)
                # alternate DVE/ACT so the partial copies keep pace with the
                # 3-matmul chains (a single engine's copy rate would stall
                # the ready phase on PSUM buffer reuse)
                if len(partials) % 2 == 0:
                    nc.vector.tensor_copy(part, o_ps)
                else:
                    nc.scalar.copy(part, o_ps)
                partials.append(part)
            for (qt_i, ecc), part in zip(tail_idx, partials):
                esl = slice(ecc * 512, (ecc + 1) * 512)
                q0 = (qc_n - 1) * 4 + qt_i
                o_ps = ps_at.tile([128, 512], F32, tag="ps")
                nc.tensor.matmul(
                    o_ps,
                    prev_atn[pairs - 1][:, qt_i * 128 : (qt_i + 1) * 128],
                    wo_sb[:, pairs - 1, esl],
                    start=True,
                    stop=True,
                )
                o_sb = outsb_pool.tile([128, 512], F32, tag="o")
                nc.vector.tensor_add(o_sb, o_ps, part)
                nc.sync.dma_start(
                    out=out.ap()[q0 * 128 : (q0 + 1) * 128, esl], in_=o_sb)


    nc.compile()
    return nc


_PROGRAM_CACHE = {}


def _get_program(key):
    if key not in _PROGRAM_CACHE:
        _PROGRAM_CACHE[key] = build_program(*key)
    return _PROGRAM_CACHE[key]


def kernel(queries, keys, values, Wq, bq, Wk, bk, Wv, bv, Wo, bo):
    global LAST_EXEC_TIME_NS
    bf16 = ml_dtypes.bfloat16

    nc = _get_program((S, D, HC, D))

    xT = {}
    for name, arr in (("q", queries), ("k", keys), ("v", values)):
        xT[name] = [
            np.ascontiguousarray(np.asarray(arr[b]).T).astype(bf16)
            for b in range(B)
        ]
    Wq, Wk, Wv, Wo = (np.asarray(w) for w in (Wq, Wk, Wv, Wo))
    bqv, bkv, bvv = (np.asarray(v, dtype=np.float32) for v in (bq, bk, bv))

    in_maps = []
    for c in range(N_CORES):
        b, g = c // 2, c % 2
        csl = slice(g * DPC, (g + 1) * DPC)
        in_maps.append(
            {
                "xqT": xT["q"][b],
                "xkT": xT["k"][b],
                "xvT": xT["v"][b],
                "wq": np.ascontiguousarray(Wq[:, csl]).astype(bf16),
                "wk": np.ascontiguousarray(Wk[:, csl]).astype(bf16),
                "wv": np.ascontiguousarray(Wv[:, csl]).astype(bf16),
                "wo": np.ascontiguousarray(Wo[csl, :]).astype(bf16),
                "bq": np.ascontiguousarray(bqv[csl]),
                "bk": np.ascontiguousarray(bkv[csl]),
            }
        )

    trace = os.environ.get("KERNEL_TRACE", "0") == "1"
    res = run_bass_kernel_spmd(nc, in_maps, list(range(N_CORES)), trace=trace)
    LAST_EXEC_TIME_NS = res.exec_time_ns

    bo = np.asarray(bo, dtype=np.float32)
    # bv commutes through attention exactly: softmax rows sum to 1, so the
    # V bias contributes the constant vector bv @ Wo to every output row.
    const = bo + bvv.astype(np.float32) @ Wo.astype(np.float32)
    out = np.empty((B, S, D), dtype=np.float32)
    for b in range(B):
        out[b] = res.results[2 * b]["out"] + res.results[2 * b + 1]["out"] + const
    return out


if __name__ == "__main__":
    rng = np.random.default_rng(0)
    t0 = time.time()
    nc = _get_program((S, D, HC, D))
    print(f"build+compile: {time.time() - t0:.1f}s")


# revision 33
# speedup vs baseline: 1.0225x; 1.0007x over previous
"""Multi-head attention (B=4, S=2048, D=1024, H=16, DK=64) on 8 TRN2 cores.

Sharding: core c = (b, g) with b = c//2 in 0..3 (data parallel on batch) and
g = c%2 (tensor parallel on heads: 8 heads / 512 d' columns per group).
Each core computes a partial output projection; the host sums the two
partials per batch (the "all-reduce" of the sharding hint, done host-side)
and adds bo + bv@Wo (the V bias commutes through attention exactly).

Per-core device algorithm (all matmul inputs bf16, fp32 PSUM accumulation):
  QT[d',q] = Wq_g^T Xq^T  (+bq via DVE per-partition add on the PSUM copy)
  KT[d',k] = same with Wk
  Vn[k,d'] = Xv Wv_g       natural layout, plus a ones column per head
                           -> V_aug [k, 65] per head (bv folded on host)
  per (q-chunk, head):
    ST[k,q]   = scores via lhsT=KT slice, rhs=QT slice (head pairs run in
                disjoint PE row groups)
    P = exp(ST/8)          one ACT pass per k-tile x 2 heads (PSUM->SBUF)
    AT_aug    = sum_k V_aug^T P  -> [65, q]: rows 0..63 = V^T P,
                                    row 64 = softmax denominators
    r = 1/AT_aug[64]       DVE reciprocal_approx_fast
    ATn = AT_aug[0:64] * r  (r broadcast via DRAM-bounce DMA in steady
                             state; via a tiny fp32 PE matmul in the tail)
  out[q,e] += sum_h ATn_h^T Wo_h   partial output projection (fp32 out)
"""

import os
import sys
import time
import types

sys.path.insert(0, "/opt/trn_rl_repo")

import numpy as np
import ml_dtypes

# ---------------------------------------------------------------------------
# axon NTFF profile hook (missing from this image's antenv stub); harmless
# when tracing is disabled.
# ---------------------------------------------------------------------------
def _install_axon_hooks():
    import antenv

    if "antenv.axon_hooks" in sys.modules:
        return
    hooks = types.ModuleType("antenv.axon_hooks")
    hooks._hook = None
    hooks.set_axon_ntff_profile_hook = lambda h: setattr(hooks, "_hook", h)
    hooks.get_axon_ntff_profile_hook = lambda: hooks._hook
    sys.modules["antenv.axon_hooks"] = hooks
    antenv.axon_hooks = hooks
    try:
        from trn_agent_boot.trn_boot import _ntff_profile_via_ctypes

        hooks.set_axon_ntff_profile_hook(
            _ntff_profile_via_ctypes("/opt/axon/libaxon_pjrt.so")
        )
    except Exception:
        pass


_install_axon_hooks()

import concourse.bacc as bacc
import concourse.bass as bass
import concourse.tile as tile
from concourse import mybir
from concourse import bass_utils
from concourse.bass_utils import run_bass_kernel_spmd

# The trace path uploads artifacts to a network bucket; keep it local.
bass_utils.upload_artifacts = lambda tmpdir: tmpdir

BF16 = mybir.dt.bfloat16
F32 = mybir.dt.float32

# Problem dims (hardcoded per spec)
B, S, D = 4, 2048, 1024
H, DK = 16, 64
N_CORES = 8
HC = H // N_CORES * B  # heads per core = 8  (16 heads / 2 groups)
DPC = HC * DK  # d' columns per core = 512

LAST_EXEC_TIME_NS = None


def build_program(s=S, dm=D, hc=HC, e=D):
    """Build the per-core Bass program. All dims in units of elements.

    s: sequence length (multiple of 512), dm: model dim (multiple of 128),
    hc: heads per core (even), e: output model dim (multiple of 512).
    """
    dk = DK
    dpc = hc * dk  # d' per core
    pairs = hc // 2
    dt_n = dm // 128  # d-tiles (contraction tiles for projections)
    st_n = s // 128  # s-tiles = k-tiles
    qc_n = s // 512  # q-chunks
    ec_n = e // 512  # out-proj column chunks

    nc = bacc.Bacc("TRN2", target_bir_lowering=False, debug=False,
                   num_devices=N_CORES)

    xqT = nc.dram_tensor("xqT", [dm, s], BF16, kind="ExternalInput")
    xkT = nc.dram_tensor("xkT", [dm, s], BF16, kind="ExternalInput")
    xvT = nc.dram_tensor("xvT", [dm, s], BF16, kind="ExternalInput")
    wq = nc.dram_tensor("wq", [dm, dpc], BF16, kind="ExternalInput")
    wk = nc.dram_tensor("wk", [dm, dpc], BF16, kind="ExternalInput")
    wv = nc.dram_tensor("wv", [dm, dpc], BF16, kind="ExternalInput")
    wo = nc.dram_tensor("wo", [dpc, e], BF16, kind="ExternalInput")
    bq = nc.dram_tensor("bq", [dpc], F32, kind="ExternalInput")
    bk = nc.dram_tensor("bk", [dpc], F32, kind="ExternalInput")
    out = nc.dram_tensor("out", [s, e], F32, kind="ExternalOutput")

    with tile.TileContext(nc) as tc:
        with (
            tc.tile_pool(name="singles", bufs=1) as singles,
            tc.tile_pool(name="xin", bufs=2) as xin,
            tc.tile_pool(name="expst", bufs=5) as expst_pool,
            tc.tile_pool(name="atn", bufs=hc + 4) as atn_pool,
            tc.tile_pool(name="small", bufs=2) as small,
            tc.tile_pool(name="outsb", bufs=3) as outsb_pool,
            tc.tile_pool(name="ps_sc", bufs=2, space="PSUM") as ps_sc,
            tc.tile_pool(name="ps_at", bufs=4, space="PSUM") as ps_at,
            tc.tile_pool(name="dramb", bufs=4, space="DRAM") as dramb,
        ):
            # ---- persistent SBUF tensors ----
            qt_sb = singles.tile([128, pairs, s], BF16, tag="qt")
            kt_sb = singles.tile([128, pairs, s], BF16, tag="kt")
            vn_sb = singles.tile([128, st_n, hc, dk + 1], BF16, tag="vn")
            wq_sb = singles.tile([128, dt_n, dpc], BF16, tag="wq")
            wk_sb = singles.tile([128, dt_n, dpc], BF16, tag="wk")
            wv_sb = singles.tile([128, dt_n, dpc], BF16, tag="wv")
            wo_sb = singles.tile([128, pairs, e], BF16, tag="wo")
            bq_sb = singles.tile([128, pairs], F32, tag="bq")
            bk_sb = singles.tile([128, pairs], F32, tag="bk")
            ones_sb = singles.tile([128, 512], BF16, tag="ones")
            ones32_sb = singles.tile([128, 64], F32, tag="ones32")

            # ones first: unblocks PE warm-up matmuls with no DMA dependency
            nc.vector.memset(ones_sb, 1.0)
            nc.vector.memset(ones32_sb, 1.0)
            # ones column of every V_aug head block
            nc.vector.memset(vn_sb[:, :, :, dk : dk + 1], 1.0)

            # PE warm-up: the HAM clock gate defaults to 1.2 GHz and takes
            # ~3.4us of sustained activity to release to 2.4 GHz. Junk
            # matmuls during the initial DMA wait warm it so the real
            # projection stream starts at full clock.
            warm_ps = ps_sc.tile([128, 1024], F32, tag="sc")
            for _ in range(24):
                nc.tensor.matmul(
                    warm_ps[:, 0:256], ones_sb[0:1, 0:128], ones_sb[0:1, 0:256],
                    start=True, stop=True,
                )

            # ---- input DMAs: wv first as 4 pieces on the sync queue (the
            # scalar queue is blocked ~1.3us by the hoisted ACT_TABLE_LOAD),
            # xv blocks on the scalar queue. The first V projection needs all
            # of wv + xv block 0; everything else has slack. bq/bk go on the
            # gpsimd SWDGE queue (many tiny descriptors, needed late).
            wv_src = wv.ap().rearrange("(t p) n -> p t n", p=128)
            for i in range(4):
                tsl = slice(2 * i, 2 * i + 2)
                nc.sync.dma_start(out=wv_sb[:, tsl, :], in_=wv_src[:, tsl, :])

            xb_v = max(1, s // 256)   # V reads 128-wide slices
            xb_p = max(1, s // 512)   # projections read 512-wide slices

            def load_x_blocked(xdram, nblk, queues=None):
                x_sb = xin.tile([128, nblk, dt_n, s // nblk], BF16, tag="x")
                src = xdram.ap().rearrange("(t p) n -> p t n", p=128)
                for j in range(nblk):
                    jsl = slice(j * s // nblk, (j + 1) * s // nblk)
                    eng = queues[j % len(queues)] if queues else nc.sync
                    eng.dma_start(out=x_sb[:, j, :, :], in_=src[:, :, jsl])
                return x_sb

            def xslice(x_sb, t, lo, width):
                nblk = x_sb.shape[1]
                bw = s // nblk
                j, off = lo // bw, lo % bw
                assert off + width <= bw
                return x_sb[:, j, t, off : off + width]

            xv_sb = load_x_blocked(xvT, xb_v, queues=[nc.scalar])
            nc.gpsimd.dma_start(
                out=bq_sb, in_=bq.ap().rearrange("(a p) -> p a", p=128))
            nc.gpsimd.dma_start(
                out=bk_sb, in_=bk.ap().rearrange("(a p) -> p a", p=128))
            nc.sync.dma_start(
                out=wk_sb, in_=wk.ap().rearrange("(t p) n -> p t n", p=128))
            xk_sb = load_x_blocked(xkT, xb_p, queues=[nc.scalar, nc.sync])
            nc.scalar.dma_start(
                out=wq_sb, in_=wq.ap().rearrange("(t p) n -> p t n", p=128))
            nc.sync.dma_start(
                out=wo_sb, in_=wo.ap().rearrange("(a p) e -> p a e", p=128))

            # ---- helper closures ----
            def proj_qk_pair(w_sb, b_sb, x_sb, dst, p, qc):
                """One [d' 128, q 512] projection chain for a head pair."""
                ps = ps_at.tile([128, 512], F32, tag="ps")
                for t in range(dt_n):
                    nc.tensor.matmul(
                        ps,
                        w_sb[:, t, p * 128 : (p + 1) * 128],
                        xslice(x_sb, t, qc * 512, 512),
                        start=(t == 0),
                        stop=(t == dt_n - 1),
                    )
                nc.vector.tensor_scalar_add(
                    dst[:, p, qc * 512 : (qc + 1) * 512], ps, b_sb[:, p : p + 1])

            # ---- stage A upfront ----
            # V first (its DMAs were issued first); most of K/Q projection is
            # deferred into B(0) as PE filler.
            vw = min(512, dpc)
            for st in range(st_n):
                for nchunk in range(dpc // vw):
                    nsl = slice(nchunk * vw, (nchunk + 1) * vw)
                    ps = ps_at.tile([128, vw], F32, tag="ps")
                    for t in range(dt_n):
                        nc.tensor.matmul(
                            ps,
                            xslice(xv_sb, t, st * 128, 128),
                            wv_sb[:, t, nsl],
                            start=(t == 0),
                            stop=(t == dt_n - 1),
                        )
                    nc.vector.tensor_copy(
                        vn_sb[
                            :, st,
                            nchunk * (vw // dk) : (nchunk + 1) * (vw // dk),
                            0:dk,
                        ],
                        ps.rearrange("p (h d) -> p h d", d=dk),
                    )

            # xq reuses xv's slot (frees after the last V matmul)
            xq_sb = load_x_blocked(xqT, xb_p, queues=[nc.sync, nc.scalar])
            # KT pairs 0..2 upfront; pair 3 + QT0 pairs 1..3 fill B(0)
            for p in range(min(3, pairs)):
                for qcc in range(qc_n):
                    proj_qk_pair(wk_sb, bk_sb, xk_sb, kt_sb, p, qcc)
            proj_qk_pair(wq_sb, bq_sb, xq_sb, qt_sb, 0, 0)

            # ---- filler generators (drip-fed PE work) ----
            def proj_qk_gen(w_sb, b_sb, x_sb, dst, p, qcc):
                """Projection chain yielding after each matmul."""
                ps = ps_at.tile([128, 512], F32, tag="ps")
                for t in range(dt_n):
                    nc.tensor.matmul(
                        ps,
                        w_sb[:, t, p * 128 : (p + 1) * 128],
                        xslice(x_sb, t, qcc * 512, 512),
                        start=(t == 0),
                        stop=(t == dt_n - 1),
                    )
                    yield
                nc.vector.tensor_scalar_add(
                    dst[:, p, qcc * 512 : (qcc + 1) * 512], ps,
                    b_sb[:, p : p + 1])
                yield

            def outproj_gen(atn_q, qcc, qt_i, ecc):
                """Out-projection sequence (pair-packed, K=128 per matmul)."""
                esl = slice(ecc * 512, (ecc + 1) * 512)
                q0 = qcc * 4 + qt_i
                o_ps = ps_at.tile([128, 512], F32, tag="ps")
                for p in range(pairs):
                    nc.tensor.matmul(
                        o_ps,
                        atn_q[p][:, qt_i * 128 : (qt_i + 1) * 128],
                        wo_sb[:, p, esl],
                        start=(p == 0),
                        stop=(p == pairs - 1),
                    )
                    yield
                o_sb = outsb_pool.tile([128, 512], F32, tag="o")
                nc.vector.tensor_copy(o_sb, o_ps)
                nc.sync.dma_start(
                    out=out.ap()[q0 * 128 : (q0 + 1) * 128, esl], in_=o_sb)
                yield

            class FillerQueue:
                def __init__(self):
                    self.tasks = []  # (gen, deadline_pr or None)

                def add(self, gen, deadline=None):
                    self.tasks.append((gen, deadline))

                def pump(self, n):
                    while n > 0 and self.tasks:
                        try:
                            next(self.tasks[0][0])
                            n -= 1
                        except StopIteration:
                            self.tasks.pop(0)

                def fence(self, pr):
                    # complete every task whose deadline is <= pr (FIFO order
                    # matches deadline order)
                    while self.tasks and any(
                        dl is not None and dl <= pr for _, dl in self.tasks
                    ):
                        self.pump(1000)

                def drain(self):
                    while self.tasks:
                        self.pump(1000)

            # ---- stages B+C interleaved over q-chunks ----
            # B(qc) processes HEAD PAIRS: the two heads' score matmuls run
            # concurrently in different PE row groups (K=64 each); one
            # [128, 1024] exp ACT covers both heads of one k-tile. Filler
            # matmuls (C(qc-1) out-projections, QT(qc+1) projections) are
            # drip-fed between groups to keep the PE dense for HAM.
            prev_atn = None
            pending_norm = None
            pending_tail_norm = []
            for qc in range(qc_n):
                qsl = slice(qc * 512, (qc + 1) * 512)
                last = qc == qc_n - 1
                atn_q = []
                rs_sb = small.tile([hc, 512], F32, tag="rs")

                fill = FillerQueue()
                if qc == 0:
                    for pp in range(1, pairs):
                        fill.add(
                            proj_qk_gen(wq_sb, bq_sb, xq_sb, qt_sb, pp, 0),
                            deadline=pp,
                        )
                        if pairs == 4 and pp < 4:
                            fill.add(
                                proj_qk_gen(wk_sb, bk_sb, xk_sb, kt_sb, 3, pp - 1),
                                deadline=3,
                            )
                    if pairs == 4:
                        fill.add(
                            proj_qk_gen(wk_sb, bk_sb, xk_sb, kt_sb, 3, 3),
                            deadline=3,
                        )
                # QT projections first: they are never gated, while the
                # out-projections of qc-1 wait on its normalization chain --
                # a stalled filler matmul blocks the whole PE FIFO.
                if qc + 1 < qc_n:
                    for pp in range(pairs):
                        fill.add(proj_qk_gen(
                            wq_sb, bq_sb, xq_sb, qt_sb, pp, qc + 1))
                if last and pending_norm is not None:
                    # no QT fillers ahead of the out-projections in the last
                    # q-chunk: the normalization must be emitted before them
                    pending_norm()
                    pending_norm = None
                # out-projection filler assignment, rebalanced so the last
                # q-chunk (which has no QT fillers) still has enough PE work
                # to stay ahead of the exp stream: C(qc0) fully in qc1,
                # C(qc1) split 5 in qc2 + 3 in qc3, C(qc2) 6 in qc3 + 2 in
                # the tail, C(qc3) in the tail.
                seq_total = 4 * ec_n
                if last and qc_n >= 3:
                    for sq in range(seq_total - 3, seq_total):
                        fill.add(outproj_gen(
                            prev_prev_atn, qc - 2, sq // ec_n, sq % ec_n))
                if prev_atn is not None:
                    if last:
                        n_seq = seq_total - 2
                    elif qc == qc_n - 2:
                        n_seq = seq_total - 3
                    else:
                        n_seq = seq_total
                    for sq in range(n_seq):
                        fill.add(outproj_gen(
                            prev_atn, qc - 1, sq // ec_n, sq % ec_n))

                def make_emit(at_A, at_B, pr):
                    def emit_at(kk, e_sb):
                        nc.tensor.matmul(
                            at_A,
                            vn_sb[:, kk, 2 * pr, :],
                            e_sb[:, 0:512],
                            start=(kk == 0),
                            stop=(kk == st_n - 1),
                        )
                        nc.tensor.matmul(
                            at_B,
                            vn_sb[:, kk, 2 * pr + 1, :],
                            e_sb[:, 512:1024],
                            start=(kk == 0),
                            stop=(kk == st_n - 1),
                        )
                    return emit_at

                def make_finalize(at_A, at_B, pr):
                    def finalize():
                        nonlocal pending_norm
                        den_dram = None
                        if last and pr == pairs - 1:
                            # final pair: no following PE work hides a DRAM
                            # bounce, so instead hop each denominator row to
                            # partition 0 (one small SBUF->SBUF DMA), take
                            # the reciprocal there, and later broadcast with
                            # a base-0 PE matmul emitted BETWEEN the tail's
                            # ready phase and its gated phase (so the matmul
                            # never head-blocks ready work). Copies run on
                            # the ACT engine (idle after the last exp).
                            rec0s = []
                            for i, at_ps in enumerate((at_A, at_B)):
                                rs_row = small.tile(
                                    [65, 512], F32, tag="rsrow")
                                nc.scalar.copy(
                                    rs_row[64:65, :], at_ps[64:65, :])
                                den0 = small.tile([1, 512], F32, tag="den0")
                                nc.scalar.dma_start(
                                    out=den0, in_=rs_row[64:65, :])
                                rec0 = small.tile([1, 512], F32, tag="rec0")
                                nc.vector.reciprocal_approx_fast(
                                    out=rec0, in_=den0)
                                rec0s.append(rec0)
                        elif last:
                            # denominator rows first: their DMA chain is the
                            # critical path of this pair's normalization, so
                            # it launches before the atn copies.
                            den_dram = dramb.tile([2, 512], F32, tag="dend")
                            for i, at_ps in enumerate((at_A, at_B)):
                                rs_row = small.tile(
                                    [65, 512], F32, tag="rsrow")
                                nc.vector.tensor_copy(
                                    rs_row[64:65, :], at_ps[64:65, :])
                                nc.scalar.dma_start(
                                    out=den_dram[i : i + 1, :],
                                    in_=rs_row[64:65, :])
                        # pair tile: head A on partitions 0-63 (direct DVE
                        # copy), head B shifted to 64-127 via SBUF->SBUF DMA
                        atn_pair = atn_pool.tile([128, 512], BF16, tag="atn")
                        nc.vector.tensor_copy(
                            atn_pair[0:64, :], at_A[0:64, :])
                        btmp = small.tile([64, 512], BF16, tag="btmp")
                        nc.vector.tensor_copy(btmp, at_B[0:64, :])
                        nc.sync.dma_start(
                            out=atn_pair[64:128, :], in_=btmp)
                        atn_q.append(atn_pair)
                        if pr == 0 and pending_norm is not None:
                            pending_norm()
                            pending_norm = None
                        if not last:
                            for h, at_ps in (
                                (2 * pr, at_A),
                                (2 * pr + 1, at_B),
                            ):
                                rs_row = small.tile(
                                    [65, 512], F32, tag="rsrow")
                                nc.vector.tensor_copy(
                                    rs_row[64:65, :], at_ps[64:65, :])
                                nc.sync.dma_start(
                                    out=rs_sb[h : h + 1, :],
                                    in_=rs_row[64:65, :])
                        elif pr == pairs - 1:
                            def tail_norm(atn_pair=atn_pair, rec0s=rec0s):
                                bc_ps = ps_at.tile(
                                    [128, 512], F32, tag="ps")
                                for i, lo in ((0, 0), (1, 64)):
                                    nc.tensor.matmul(
                                        bc_ps[lo : lo + 64, :],
                                        ones32_sb[0:1, 0:64],
                                        rec0s[i],
                                        start=True,
                                        stop=True,
                                    )
                                nc.vector.tensor_mul(
                                    atn_pair, atn_pair, bc_ps)
                            pending_tail_norm.append(tail_norm)
                        else:
                            # per-pair normalization, pipelined under later
                            # pairs and kept entirely off the PE FIFO (a PE
                            # instruction waiting on this chain would
                            # head-block every later matmul). Broadcast the
                            # DENOMINATORS via a DRAM bounce into a [128,512]
                            # pair tile, then one base-0
                            # reciprocal_approx_fast + one gpsimd multiply
                            # cover both heads.
                            bc_sb = small.tile([128, 512], F32, tag="bc")
                            for i, lo in ((0, 0), (1, 64)):
                                row = den_dram[i : i + 1, :]
                                bcast_src = bass.AP(
                                    tensor=row.tensor,
                                    offset=row.offset,
                                    ap=[[0, 64]] + list(row.ap[1:]),
                                )
                                nc.sync.dma_start(
                                    out=bc_sb[lo : lo + 64, :],
                                    in_=bcast_src)
                            rec_bc = small.tile([128, 512], F32, tag="recbc",
                                                bufs=1)
                            nc.vector.reciprocal_approx_fast(
                                out=rec_bc, in_=bc_sb)
                            nc.gpsimd.tensor_mul(atn_pair, atn_pair, rec_bc)
                    return finalize

                # AT matmuls trail their exp by two k-tiles and flush into
                # the NEXT pair's first iterations, so no AT ever waits on a
                # just-issued exp (the old per-pair flush exposed one
                # exp-latency stall per pair).
                pipe = []  # (emit_fn, kk, e_sb, finalize_or_None)

                def flush_one():
                    emit_fn, kk2, e_sb2, fin = pipe.pop(0)
                    emit_fn(kk2, e_sb2)
                    if fin is not None:
                        fin()

                for pr in range(pairs):
                    fill.fence(pr)
                    at_A = ps_at.tile([65, 512], F32, tag="ps")
                    at_B = ps_at.tile([65, 512], F32, tag="ps")
                    emit_fn = make_emit(at_A, at_B, pr)
                    fin_fn = make_finalize(at_A, at_B, pr)

                    for kk in range(st_n):
                        sc_ps = ps_sc.tile([128, 1024], F32, tag="sc")
                        ksl = slice(kk * 128, (kk + 1) * 128)
                        nc.tensor.matmul(
                            sc_ps[:, 0:512],
                            kt_sb[0:64, pr, ksl],
                            qt_sb[0:64, pr, qsl],
                            start=True,
                            stop=True,
                        )
                        nc.tensor.matmul(
                            sc_ps[:, 512:1024],
                            kt_sb[64:128, pr, ksl],
                            qt_sb[64:128, pr, qsl],
                            start=True,
                            stop=True,
                        )
                        exp_sb = expst_pool.tile([128, 1024], BF16, tag="e")
                        nc.scalar.activation(
                            exp_sb, sc_ps,
                            mybir.ActivationFunctionType.Exp,
                            scale=1.0 / np.sqrt(dk),
                        )
                        if len(pipe) >= 2:
                            flush_one()
                        pipe.append((
                            emit_fn, kk, exp_sb,
                            fin_fn if kk == st_n - 1 else None,
                        ))
                        if last and pr == 0 and kk < 4:
                            pass  # let the qc-2 norm chain land before its
                                  # out-projections can head-block the FIFO
                        elif kk % 2 == 1:
                            # bursts of 2: consecutive filler matmuls from
                            # the same chain pipeline their LDWEIGHTS
                            fill.pump(4 if last or pr == pairs - 1 else 2)

                while pipe:
                    flush_one()
                fill.drain()

                # batched softmax normalization for the whole q-chunk; its
                # emission is deferred into the next q-chunk (after the first
                # pair unit) so the DVE/DMA latency chain never head-blocks
                # the FIFO right at the q-chunk boundary.
                if not last:
                    def make_norm(rs_sb=rs_sb, atn_q=atn_q):
                        def norm():
                            rec_sb = small.tile([hc, 512], F32, tag="rec",
                                                bufs=1)
                            nc.vector.reciprocal_approx_fast(
                                out=rec_sb, in_=rs_sb)
                            rec_dram = dramb.tile([hc, 512], F32, tag="recd")
                            nc.sync.dma_start(out=rec_dram, in_=rec_sb)
                            # one [128,512] multiply per PAIR (both heads'
                            # broadcast rows packed in one tile): halves the
                            # gpsimd multiply count vs per-head multiplies
                            for p in range(pairs):
                                bc_sb = small.tile([128, 512], F32, tag="bc")
                                for j, lo in ((0, 0), (1, 64)):
                                    h = 2 * p + j
                                    row = rec_dram[h : h + 1, :]
                                    bcast_src = bass.AP(
                                        tensor=row.tensor,
                                        offset=row.offset,
                                        ap=[[0, 64]] + list(row.ap[1:]),
                                    )
                                    nc.sync.dma_start(
                                        out=bc_sb[lo : lo + 64, :],
                                        in_=bcast_src)
                                nc.gpsimd.tensor_mul(
                                    atn_q[p], atn_q[p], bc_sb)
                        return norm
                    pending_norm = make_norm()
                prev_prev_atn = prev_atn
                prev_atn = atn_q

            # tail: the deferred C(qc_n-2) sequences are fully ready; run
            # them first. The C(qc_n-1) sequences split into a READY phase
            # (pairs 0..2, whose norms landed during the last q-chunk;
            # partial sums parked in SBUF) and a GATED phase (one pair-3
            # matmul + DVE add each). All 24 ready matmuls emit before any
            # instruction gated on the final pair's normalization chain, so
            # the PE stays busy under it and only ~2us of gated work
            # remains at the very end.
            if qc_n >= 2:
                for sq in range(4 * ec_n - 2, 4 * ec_n):
                    for _ in outproj_gen(
                        prev_prev_atn, qc_n - 2, sq // ec_n, sq % ec_n
                    ):
                        pass
            tail_idx = [(qt_i, ecc) for qt_i in range(4) for ecc in range(ec_n)]
            partials = []
            for qt_i, ecc in tail_idx:
                esl = slice(ecc * 512, (ecc + 1) * 512)
                o_ps = ps_at.tile([128, 512], F32, tag="ps")
                for p in range(pairs - 1):
                    nc.tensor.matmul(
                        o_ps,
                        prev_atn[p][:, qt_i * 128 : (qt_i + 1) * 128],
                        wo_sb[:, p, esl],
                        start=(p == 0),
                        stop=(p == pairs - 2),
                    )
                part = outsb_pool.tile([128, 512], BF16, tag="part", bufs=8)
                # partial copies on ACT (idle after the last exp): the DVE's
                # stream is head-blocked by the final-pair reciprocals
                # waiting on their partition-0 hops
                nc.scalar.copy(part, o_ps)
                partials.append(part)
            for fn in pending_tail_norm:
                fn()
            for (qt_i, ecc), part in zip(tail_idx, partials):
                esl = slice(ecc * 512, (ecc + 1) * 512)
                q0 = (qc_n - 1) * 4 + qt_i
                o_ps = ps_at.tile([128, 512], F32, tag="ps")
                nc.tensor.matmul(
                    o_ps,
                    prev_atn[pairs - 1][:, qt_i * 128 : (qt_i + 1) * 128],
                    wo_sb[:, pairs - 1, esl],
                    start=True,
                    stop=True,
                )
                o_sb = outsb_pool.tile([128, 512], F32, tag="o")
                nc.vector.tensor_add(o_sb, o_ps, part)
                nc.sync.dma_start(
                    out=out.ap()[q0 * 128 : (q0 + 1) * 128, esl], in_=o_sb)


    nc.compile()
    return nc


_PROGRAM_CACHE = {}


def _get_program(key):
    if key not in _PROGRAM_CACHE:
        _PROGRAM_CACHE[key] = build_program(*key)
    return _PROGRAM_CACHE[key]


def kernel(queries, keys, values, Wq, bq, Wk, bk, Wv, bv, Wo, bo):
    global LAST_EXEC_TIME_NS
    bf16 = ml_dtypes.bfloat16

    nc = _get_program((S, D, HC, D))

    xT = {}
    for name, arr in (("q", queries), ("k", keys), ("v", values)):
        xT[name] = [
            np.ascontiguousarray(np.asarray(arr[b]).T).astype(bf16)
            for b in range(B)
        ]
    Wq, Wk, Wv, Wo = (np.asarray(w) for w in (Wq, Wk, Wv, Wo))
    bqv, bkv, bvv = (np.asarray(v, dtype=np.float32) for v in (bq, bk, bv))

    in_maps = []
    for c in range(N_CORES):
        b, g = c // 2, c % 2
        csl = slice(g * DPC, (g + 1) * DPC)
        in_maps.append(
            {
                "xqT": xT["q"][b],
                "xkT": xT["k"][b],
                "xvT": xT["v"][b],
                "wq": np.ascontiguousarray(Wq[:, csl]).astype(bf16),
                "wk": np.ascontiguousarray(Wk[:, csl]).astype(bf16),
                "wv": np.ascontiguousarray(Wv[:, csl]).astype(bf16),
                "wo": np.ascontiguousarray(Wo[csl, :]).astype(bf16),
                "bq": np.ascontiguousarray(bqv[csl]),
                "bk": np.ascontiguousarray(bkv[csl]),
            }
        )

    trace = os.environ.get("KERNEL_TRACE", "0") == "1"
    res = run_bass_kernel_spmd(nc, in_maps, list(range(N_CORES)), trace=trace)
    LAST_EXEC_TIME_NS = res.exec_time_ns

    bo = np.asarray(bo, dtype=np.float32)
    # bv commutes through attention exactly: softmax rows sum to 1, so the
    # V bias contributes the constant vector bv @ Wo to every output row.
    const = bo + bvv.astype(np.float32) @ Wo.astype(np.float32)
    out = np.empty((B, S, D), dtype=np.float32)
    for b in range(B):
        out[b] = res.results[2 * b]["out"] + res.results[2 * b + 1]["out"] + const
    return out


if __name__ == "__main__":
    rng = np.random.default_rng(0)
    t0 = time.time()
    nc = _get_program((S, D, HC, D))
    print(f"build+compile: {time.time() - t0:.1f}s")
